# revision 1
# baseline (speedup 1.0000x reference)
"""Single-head attention (B=4, S=2048, D=E=1024) on 8 trn2 NeuronCores.

Sharding: data-parallel over (batch, q-half) -> 8 shards. Each core gets a
1024-row q shard plus the full 2048 keys of its batch; K/V projections are
recomputed on both cores of a batch pair (25% extra flops, zero collectives).

Per-core math (all "T" tensors are token-transposed on the host so that the
contraction dim lands on SBUF partitions; no on-device transposes needed):
  qp^T [E,q]   = (lhsT=wq[D,E], rhs=qT[D,q]) * (1/sqrt E) + bq/sqrt(E)
  kp^T [E,k]   = (lhsT=wk, rhs=kT) + bk
  vp   [k,E]   = (lhsT=vT[D,k], rhs=wv[D,E]) + bv
  lgT  [k,q]   = (lhsT=kp^T slice, rhs=qp^T)            (scale folded into qp)
  expT [k,q]   = Exp(lgT + mask*NEG)                    (ACT, per-partition bias)
  s    [.,q]   = ones-matmul over expT                  (softmax sum; no max-sub:
                                                         logits ~ N(0,1), safe)
  ctx^T[E,q]   = (lhsT=vp slice, rhs=expT) * recip(s)
  out  [q,D]   = (lhsT=ctx^T slice, rhs=ow[E,D]) + ob
All matmuls run as float32r (full PE rate at N>=256), fp32 data + accumulate.
Pool lifetimes follow strict LIFO (Tile pool-stack requirement).
"""

import os
import numpy as np

P = 128
NEG = -1.0e9


def build_nc(D=1024, E=1024, SK=2048, QSH=1024, QB=512):
    """Build the per-core Bass module (SPMD; same program on all cores)."""
    import concourse.bass as bass
    import concourse.mybir as mybir
    import concourse.tile as tile
    from concourse import bacc

    f32 = mybir.dt.float32
    f32r = mybir.dt.float32r
    AF = mybir.ActivationFunctionType

    DT = D // P          # contraction tiles over model dim
    ET = E // P          # enc tiles
    KT = SK // P         # key tiles
    NQB = QSH // QB      # q blocks
    KNB = min(512, SK)   # key free-dim block for kp
    ENB = min(512, E)    # E free-dim block for vp
    DNB = min(512, D)    # model free-dim block for out
    DTH = max(1, DT // 2)  # split-K half for kp streaming
    ISCALE = 1.0 / float(np.sqrt(E))

    nc = bacc.Bacc(trn_type="TRN2")

    # ---- I/O ----
    qT = nc.dram_tensor("qT", [D, QSH], f32r, kind="ExternalInput")[:, :]
    kT = nc.dram_tensor("kT", [D, SK], f32r, kind="ExternalInput")[:, :]
    vT = nc.dram_tensor("vT", [D, SK], f32r, kind="ExternalInput")[:, :]
    mask_cols = nc.dram_tensor("mask_cols", [P, KT], f32, kind="ExternalInput")[:, :]
    ones_d = nc.dram_tensor("ones_d", [P, P], f32r, kind="ExternalInput")[:, :]
    wq = nc.dram_tensor("wq", [D, E], f32r, kind="ExternalInput")[:, :]
    wk = nc.dram_tensor("wk", [D, E], f32r, kind="ExternalInput")[:, :]
    wv = nc.dram_tensor("wv", [D, E], f32r, kind="ExternalInput")[:, :]
    ow = nc.dram_tensor("ow", [E, D], f32r, kind="ExternalInput")[:, :]
    bq_col = nc.dram_tensor("bq_col", [P, ET], f32, kind="ExternalInput")[:, :]
    bk_col = nc.dram_tensor("bk_col", [P, ET], f32, kind="ExternalInput")[:, :]
    bv_bc = nc.dram_tensor("bv_bc", [P, E], f32, kind="ExternalInput")[:, :]
    ob_bc = nc.dram_tensor("ob_bc", [P, D], f32, kind="ExternalInput")[:, :]
    out = nc.dram_tensor("out", [QSH, D], f32, kind="ExternalOutput")[:, :]

    qT_r = qT.rearrange("(t p) n -> p t n", p=P)   # [128, DT, QSH]
    kT_r = kT.rearrange("(t p) n -> p t n", p=P)
    vT_r = vT.rearrange("(t p) n -> p t n", p=P)
    wq_r = wq.rearrange("(t p) n -> p t n", p=P)   # [128, DT, E]
    wk_r = wk.rearrange("(t p) n -> p t n", p=P)
    wv_r = wv.rearrange("(t p) n -> p t n", p=P)
    ow_r = ow.rearrange("(t p) n -> p t n", p=P)   # [128, ET, D]

    def mm(ps, lhsT, rhs, start, stop):
        nc.tensor.matmul(ps, lhsT, rhs, start=start, stop=stop)

    with tile.TileContext(nc) as tc:
        # ---- persistent smalls (incl. per-qb softmax reciprocal + out bias) ----
        smalls_cm = tc.tile_pool(name="smalls", bufs=1)
        smalls = smalls_cm.__enter__()
        ones_t = smalls.tile([P, P], f32r, name="ones")
        nc.gpsimd.dma_start(ones_t[:], ones_d)
        mask_t = smalls.tile([P, KT], f32, name="maskc")
        nc.gpsimd.dma_start(mask_t[:], mask_cols)
        nc.scalar.mul(mask_t[:], mask_t[:], NEG)
        bq_t = smalls.tile([P, ET], f32, name="bqc")
        nc.gpsimd.dma_start(bq_t[:], bq_col)
        nc.scalar.mul(bq_t[:], bq_t[:], ISCALE)
        bk_t = smalls.tile([P, ET], f32, name="bkc")
        nc.gpsimd.dma_start(bk_t[:], bk_col)
        recip_ts = [smalls.tile([P, QB], f32, name=f"recip{i}")
                    for i in range(NQB)]

        dram_cm = tc.tile_pool(name="dramscratch", bufs=1, space="DRAM")
        dram_pool = dram_cm.__enter__()

        # ---- phase VP (first: vp outlives kp): vp [SK, E] + bv ----
        vp_cm = tc.tile_pool(name="vp", bufs=1)
        vp_pool = vp_cm.__enter__()
        vp = vp_pool.tile([P, KT, E], f32r, name="vp")
        with tc.tile_pool(name="vp_w", bufs=1) as phw, \
             tc.tile_pool(name="vp_ph", bufs=3) as ph, \
             tc.tile_pool(name="vp_ps", bufs=4, space="PSUM") as php:
            wv_t = phw.tile([P, DT, E], f32r, name="wv_t")
            NH = 2 if E >= 512 else 1
            for h in range(NH):
                for t in range(DT):
                    nc.sync.dma_start(wv_t[:, t, h * E // NH:(h + 1) * E // NH],
                                      wv_r[:, t, h * E // NH:(h + 1) * E // NH])
            bv_t = phw.tile([P, E], f32, name="bv_t")
            nc.sync.dma_start(bv_t[:], bv_bc)
            for m in range(KT):
                lhs_t = ph.tile([P, DT, P], f32r, tag="vT_s", name=f"vT_{m}")
                hh = max(1, DT // 2)
                nc.scalar.dma_start(lhs_t[:, :hh, :],
                                    vT_r[:, :hh, m * P:(m + 1) * P])
                nc.gpsimd.dma_start(lhs_t[:, hh:, :],
                                    vT_r[:, hh:, m * P:(m + 1) * P])
                for n in range(E // ENB):
                    ps = php.tile([P, ENB], f32, tag="ps", name=f"vpps_{m}_{n}")
                    for t in range(DT):
                        mm(ps[:], lhs_t[:, t, :],
                           wv_t[:, t, n * ENB:(n + 1) * ENB],
                           t == 0, t == DT - 1)
                    nc.vector.tensor_add(vp[:, m, n * ENB:(n + 1) * ENB], ps[:],
                                         bv_t[:, n * ENB:(n + 1) * ENB])

        # ---- phase KP: kp^T [E, SK] + bk (kT streamed in split-K halves) ----
        kp_cm = tc.tile_pool(name="kp", bufs=1)
        kp_pool = kp_cm.__enter__()
        kp = kp_pool.tile([P, ET, SK], f32r, name="kp")
        with tc.tile_pool(name="kp_w", bufs=1) as phw, \
             tc.tile_pool(name="kp_ph", bufs=3) as ph, \
             tc.tile_pool(name="kp_ps", bufs=1, space="PSUM") as php:
            wk_t = phw.tile([P, DT, E], f32r, name="wk_t")
            for h in range(2):
                for t in range(DT):
                    eng = nc.sync if t % 2 == 0 else nc.scalar
                    eng.dma_start(wk_t[:, t, h * E // 2:(h + 1) * E // 2],
                                  wk_r[:, t, h * E // 2:(h + 1) * E // 2])
            for n in range(SK // KNB):
                pss = [php.tile([P, KNB], f32, tag=f"ps{m}", name=f"kpps_{n}_{m}")
                       for m in range(ET)]
                for th in range(DT // DTH):
                    rhs_t = ph.tile([P, DTH, KNB], f32r, tag="kT_s",
                                    name=f"kT_{n}_{th}")
                    for ti in range(DTH):
                        t = th * DTH + ti
                        eng = nc.gpsimd
                        eng.dma_start(rhs_t[:, ti, :],
                                      kT_r[:, t, n * KNB:(n + 1) * KNB])
                    for m in range(ET):
                        for ti in range(DTH):
                            t = th * DTH + ti
                            mm(pss[m][:], wk_t[:, t, m * P:(m + 1) * P],
                               rhs_t[:, ti, :], t == 0, t == DT - 1)
                for m in range(ET):
                    nc.scalar.activation(kp[:, m, n * KNB:(n + 1) * KNB],
                                         pss[m][:], AF.Identity,
                                         bias=bk_t[:, m:m + 1])

        # ---- attention per q-block ----
        ctx_bounce = []
        ctx_last = None
        ctx_last_cm = None

        for qb in range(NQB):
            q0 = qb * QB
            last_qb = qb == NQB - 1

            exp_cm = tc.tile_pool(name=f"exp{qb}", bufs=1)
            exp_pool = exp_cm.__enter__()
            expT = exp_pool.tile([P, KT, QB], f32r, name=f"exp{qb}")

            # -- prologue: qp^T for this q block --
            qp_cm = tc.tile_pool(name=f"qp{qb}", bufs=1)
            qp_pool = qp_cm.__enter__()
            qp = qp_pool.tile([P, ET, QB], f32r, name=f"qp{qb}")
            with tc.tile_pool(name=f"qpro{qb}", bufs=2) as ph, \
                 tc.tile_pool(name=f"qpro_ps{qb}", bufs=1, space="PSUM") as php:
                pss = [php.tile([P, QB], f32, tag=f"ps{m}", name=f"qpps{qb}_{m}")
                       for m in range(ET)]
                for t in range(DT):
                    wq_t = ph.tile([P, E], f32r, tag="wq_s", name=f"wq{qb}_{t}")
                    for h in range(2):
                        eng = nc.sync if h == 0 else nc.scalar
                        eng.dma_start(wq_t[:, h * E // 2:(h + 1) * E // 2],
                                      wq_r[:, t, h * E // 2:(h + 1) * E // 2])
                    qt_t = ph.tile([P, QB], f32r, tag="qT_s", name=f"qt{qb}_{t}")
                    nc.scalar.dma_start(qt_t[:], qT_r[:, t, q0:q0 + QB])
                    for m in range(ET):
                        mm(pss[m][:], wq_t[:, m * P:(m + 1) * P], qt_t[:],
                           t == 0, t == DT - 1)
                for m in range(ET):
                    nc.scalar.activation(qp[:, m, :], pss[m][:], AF.Identity,
                                         bias=bq_t[:, m:m + 1], scale=ISCALE)

            # -- logits + exp + softmax sum --
            with tc.tile_pool(name=f"lg_ps{qb}", bufs=4, space="PSUM") as php, \
                 tc.tile_pool(name=f"s_ps{qb}", bufs=1, space="PSUM") as sphp:
                s_ps = sphp.tile([P, QB], f32, name=f"sps{qb}")
                for kb in range(KT):
                    ps = php.tile([P, QB], f32, tag="ps", name=f"lgps{qb}_{kb}")
                    for e in range(ET):
                        mm(ps[:], kp[:, e, kb * P:(kb + 1) * P], qp[:, e, :],
                           e == 0, e == ET - 1)
                    nc.scalar.activation(expT[:, kb, :], ps[:], AF.Exp,
                                         bias=mask_t[:, kb:kb + 1])
                    mm(s_ps[:], ones_t[:], expT[:, kb, :], kb == 0, kb == KT - 1)
                nc.vector.reciprocal(recip_ts[qb][:], s_ps[:])

            qp_cm.__exit__(None, None, None)  # qp dead after logits

            # -- ctx accumulation --
            ctx_ps_cm = tc.tile_pool(name=f"ctx_ps{qb}", bufs=1, space="PSUM")
            ctx_php = ctx_ps_cm.__enter__()
            cps = [ctx_php.tile([P, QB], f32, tag=f"ps{e}", name=f"ctxps{qb}_{e}")
                   for e in range(ET)]
            for e in range(ET):
                for kb in range(KT):
                    mm(cps[e][:], vp[:, kb, e * P:(e + 1) * P], expT[:, kb, :],
                       kb == 0, kb == KT - 1)

            exp_cm.__exit__(None, None, None)  # expT consumed
            if last_qb:
                kp_cm.__exit__(None, None, None)  # kp dead after last logits

            # -- normalize into SBUF ctx^T --
            ctxs_cm = tc.tile_pool(name=f"ctxs{qb}", bufs=1)
            ctxs_pool = ctxs_cm.__enter__()
            ctx_sb = ctxs_pool.tile([P, ET, QB], f32r, name=f"ctx{qb}")
            for e in range(ET):
                nc.vector.tensor_mul(ctx_sb[:, e, :], cps[e][:], recip_ts[qb][:])
            ctx_ps_cm.__exit__(None, None, None)

            if not last_qb:
                dt_ = dram_pool.tile([P, ET, QB], f32r, name=f"ctxd{qb}")
                for e in range(ET):
                    nc.gpsimd.dma_start(dt_[:, e, :], ctx_sb[:, e, :])
                ctx_bounce.append(dt_)
                ctxs_cm.__exit__(None, None, None)
            else:
                ctx_bounce.append(None)
                ctx_last = ctx_sb
                ctx_last_cm = ctxs_cm

        # ---- out phase: out[q, :] = ctx @ ow + ob (ow streamed small) ----
        with tc.tile_pool(name="ctx_back", bufs=1) as cb, \
             tc.tile_pool(name="ow_s", bufs=8) as ows, \
             tc.tile_pool(name="outsb", bufs=6) as osb, \
             tc.tile_pool(name="out_ps", bufs=1, space="PSUM") as php:
            ob_t = cb.tile([P, D], f32, name="ob_t")
            nc.sync.dma_start(ob_t[:], ob_bc)
            ctx_ts = []
            for qb in range(NQB):
                if ctx_bounce[qb] is not None:
                    ctx_t = cb.tile([P, ET, QB], f32r, tag=f"cback{qb}",
                                    name=f"cb{qb}")
                    for e in range(ET):
                        nc.gpsimd.dma_start(ctx_t[:, e, :], ctx_bounce[qb][:, e, :])
                    ctx_ts.append(ctx_t)
                else:
                    ctx_ts.append(ctx_last)
            MQ = QB // P
            for nd in range(D // DNB):
                pss = {}
                for qb in range(NQB):
                    for mq in range(MQ):
                        pss[(qb, mq)] = php.tile(
                            [P, DNB], f32, tag=f"ps{qb}_{mq}",
                            name=f"ops{nd}_{qb}_{mq}")
                for e in range(ET):
                    ow_t = ows.tile([P, DNB], f32r, tag="ow_s",
                                    name=f"ow{nd}_{e}")
                    nc.sync.dma_start(ow_t[:],
                                      ow_r[:, e, nd * DNB:(nd + 1) * DNB])
                    for qb in range(NQB):
                        for mq in range(MQ):
                            mm(pss[(qb, mq)][:],
                               ctx_ts[qb][:, e, mq * P:(mq + 1) * P],
                               ow_t[:], e == 0, e == ET - 1)
                for qb in range(NQB):
                    for mq in range(MQ):
                        ot = osb.tile([P, DNB], f32, tag="ot",
                                      name=f"ot{nd}_{qb}_{mq}")
                        nc.vector.tensor_add(ot[:], pss[(qb, mq)][:],
                                             ob_t[:, nd * DNB:(nd + 1) * DNB])
                        nc.gpsimd.dma_start(
                            out[qb * QB + mq * P: qb * QB + (mq + 1) * P,
                                nd * DNB:(nd + 1) * DNB], ot[:])

        if ctx_last_cm is not None:
            ctx_last_cm.__exit__(None, None, None)
        vp_cm.__exit__(None, None, None)
        dram_cm.__exit__(None, None, None)
        smalls_cm.__exit__(None, None, None)

    nc.compile()
    return nc


def make_in_maps(v, k, q, mask, wq_w, wq_b, wk_w, wk_b, wv_w, wv_b, out_w, out_b,
                 n_cores=8, D=1024, E=1024, SK=2048, QSH=1024):
    """Host-side shard + layout prep (pure data movement, no math)."""
    ET = E // P
    KT = SK // P
    f = np.float32
    wq_w = np.ascontiguousarray(np.asarray(wq_w, f))
    wk_w = np.ascontiguousarray(np.asarray(wk_w, f))
    wv_w = np.ascontiguousarray(np.asarray(wv_w, f))
    out_w = np.ascontiguousarray(np.asarray(out_w, f))
    bq_col = np.ascontiguousarray(np.asarray(wq_b, f).reshape(ET, P).T)
    bk_col = np.ascontiguousarray(np.asarray(wk_b, f).reshape(ET, P).T)
    bv_bc = np.ascontiguousarray(np.broadcast_to(np.asarray(wv_b, f), (P, E)))
    ob_bc = np.ascontiguousarray(
        np.broadcast_to(np.asarray(out_b, f), (P, len(out_b))))
    ones_arr = np.ones((P, P), f)
    in_maps = []
    for c in range(n_cores):
        b, h = divmod(c, 2)
        qTc = np.ascontiguousarray(np.asarray(q[b, h * QSH:(h + 1) * QSH, :], f).T)
        kTc = np.ascontiguousarray(np.asarray(k[b], f).T)
        vTc = np.ascontiguousarray(np.asarray(v[b], f).T)
        mc = np.ascontiguousarray(np.asarray(mask[b, 0], f).reshape(KT, P).T)
        in_maps.append(dict(qT=qTc, kT=kTc, vT=vTc, mask_cols=mc,
                            ones_d=ones_arr,
                            wq=wq_w, wk=wk_w, wv=wv_w, ow=out_w,
                            bq_col=bq_col, bk_col=bk_col,
                            bv_bc=bv_bc, ob_bc=ob_bc))
    return in_maps


_NC_CACHE = {}


def kernel(v, k, q, mask, wq_w, wq_b, wk_w, wk_b, wv_w, wv_b, out_w, out_b):
    from concourse.bass_utils import run_bass_kernel_spmd

    B, S, D = 4, 2048, 1024
    E, QSH = 1024, 1024
    if "nc" not in _NC_CACHE:
        _NC_CACHE["nc"] = build_nc(D=D, E=E, SK=S, QSH=QSH, QB=512)
    nc = _NC_CACHE["nc"]

    in_maps = make_in_maps(v, k, q, mask, wq_w, wq_b, wk_w, wk_b, wv_w, wv_b,
                           out_w, out_b, n_cores=8, D=D, E=E, SK=S, QSH=QSH)
    trace = bool(int(os.environ.get("BASS_KERNEL_TRACE", "0")))
    res = run_bass_kernel_spmd(nc, in_maps, core_ids=list(range(8)), trace=trace)
    if trace:
        print(f"HW exec time: {res.exec_time_ns} ns")
        _NC_CACHE["last_exec_time_ns"] = res.exec_time_ns
        _NC_CACHE["last_trace"] = res.instructions_and_trace

    outp = np.empty((B, S, D), np.float32)
    for c in range(8):
        b, h = divmod(c, 2)
        outp[b, h * QSH:(h + 1) * QSH, :] = res.results[c]["out"]
    return outp



# revision 4
# speedup vs baseline: 1.1641x; 1.1641x over previous
"""Single-head attention (B=4, S=2048, D=E=1024) on 8 trn2 NeuronCores.

Sharding: data-parallel over (batch, q-half) -> 8 shards. Each core gets a
1024-row q shard plus the full 2048 keys of its batch; K/V projections are
recomputed on both cores of a batch pair (no collectives; TimelineSim does
not model remote DMA and modeled collectives run at <=40GB/s, so the
duplicated 2x2.1 GF is cheaper than any exchange).

All matmul operands are bf16 (host-converted): same PE rate as fp32r
(1 cycle/row) but half the DMA traffic and SBUF footprint, which lets every
weight stay resident and keeps the PE fed continuously. PSUM accumulation
stays fp32.

Per-core math (token-transposed on host; contraction dim on partitions):
  vp   [k,E]   = (lhsT=vT[D,k], rhs=wv[D,E])            (bv folded into ob!)
  kp^T [E,k]   = (lhsT=wk[D,e], rhs=kT[D,k]) + bk
  qp^T [E,q]   = (lhsT=wq[D,e], rhs=qT[D,q]) * (1/sqrt E) + bq/sqrt(E)
  lgT  [k,q]   = (lhsT=kp^T slice, rhs=qp^T)
  expT [k,q]   = Exp(lgT + mask*NEG)                    (ACT per-partition bias)
  s    [.,q]   = ones-matmul over expT                  (no max-sub: lg~N(0,1))
  ctx^T[E,q]   = (lhsT=vp slice, rhs=expT) * recip(s)   (DVE drain)
  out  [q,D]   = (lhsT=ctx^T slice, rhs=ow[E,D]) + ob_eff
where ob_eff = out_b + wv_b @ out_w (host-folded: softmax rows sum to 1, so
the vp bias contributes exactly bv @ ow to every output row).

The output projection runs fused inside each q-block (no ctx DRAM bounce).
"""

import numpy as np
import ml_dtypes

P = 128
NEG = -1.0e9
BF16 = np.dtype(ml_dtypes.bfloat16)


def build_nc(D=1024, E=1024, SK=2048, QSH=1024, QB=512):
    """Build the per-core Bass module (SPMD; same program on all cores)."""
    import concourse.bass as bass
    import concourse.mybir as mybir
    import concourse.tile as tile
    from concourse import bacc

    f32 = mybir.dt.float32
    bf16 = mybir.dt.bfloat16
    AF = mybir.ActivationFunctionType

    DT = D // P          # contraction tiles over model dim (8)
    ET = E // P          # enc tiles (8)
    KT = SK // P         # key tiles (16)
    NQB = QSH // QB      # q blocks (2)
    KC = 512             # key free-dim chunk for kp
    NKC = SK // KC       # 4
    DNB = 512            # model free-dim chunk for out
    MQ = QB // P         # q sub-tiles per block (4)
    ISCALE = 1.0 / float(np.sqrt(E))

    nc = bacc.Bacc(trn_type="TRN2")

    # ---- I/O ----
    qT = nc.dram_tensor("qT", [D, QSH], bf16, kind="ExternalInput")[:, :]
    kT = nc.dram_tensor("kT", [D, SK], bf16, kind="ExternalInput")[:, :]
    vT = nc.dram_tensor("vT", [D, SK], bf16, kind="ExternalInput")[:, :]
    mask_cols = nc.dram_tensor("mask_cols", [P, KT], f32, kind="ExternalInput")[:, :]
    ones_d = nc.dram_tensor("ones_d", [P, P], bf16, kind="ExternalInput")[:, :]
    wq = nc.dram_tensor("wq", [D, E], bf16, kind="ExternalInput")[:, :]
    wk = nc.dram_tensor("wk", [D, E], bf16, kind="ExternalInput")[:, :]
    wv = nc.dram_tensor("wv", [D, E], bf16, kind="ExternalInput")[:, :]
    ow = nc.dram_tensor("ow", [E, D], bf16, kind="ExternalInput")[:, :]
    bq_col = nc.dram_tensor("bq_col", [P, ET], f32, kind="ExternalInput")[:, :]
    bk_col = nc.dram_tensor("bk_col", [P, ET], f32, kind="ExternalInput")[:, :]
    ob_bc = nc.dram_tensor("ob_bc", [P, D], f32, kind="ExternalInput")[:, :]
    out = nc.dram_tensor("out", [QSH, D], f32, kind="ExternalOutput")[:, :]

    qT_r = qT.rearrange("(t p) n -> p t n", p=P)   # [128, DT, QSH]
    kT_r = kT.rearrange("(t p) n -> p t n", p=P)
    vT_r = vT.rearrange("(t p) n -> p t n", p=P)
    wq_r = wq.rearrange("(t p) n -> p t n", p=P)   # [128, DT, E]
    wk_r = wk.rearrange("(t p) n -> p t n", p=P)
    wv_r = wv.rearrange("(t p) n -> p t n", p=P)
    ow_r = ow.rearrange("(t p) n -> p t n", p=P)   # [128, ET, D]

    def mm(ps, lhsT, rhs, start, stop):
        nc.tensor.matmul(ps, lhsT, rhs, start=start, stop=stop)

    with tile.TileContext(nc) as tc:
        # ---- persistent smalls ----
        with tc.tile_pool(name="smalls", bufs=1) as smalls, \
             tc.tile_pool(name="weights", bufs=1) as wpool, \
             tc.tile_pool(name="bigres", bufs=1) as bigres:
            mask_t = smalls.tile([P, KT], f32, name="maskc")
            nc.gpsimd.dma_start(mask_t[:], mask_cols)
            bq_t = smalls.tile([P, ET], f32, name="bqc")
            nc.gpsimd.dma_start(bq_t[:], bq_col)
            bk_t = smalls.tile([P, ET], f32, name="bkc")
            nc.gpsimd.dma_start(bk_t[:], bk_col)
            ones_t = smalls.tile([P, P], bf16, name="ones")
            nc.gpsimd.dma_start(ones_t[:], ones_d)
            ob_t = smalls.tile([P, D], f32, name="ob_t")
            nc.gpsimd.dma_start(ob_t[:], ob_bc)
            recip_ts = [smalls.tile([P, QB], f32, name=f"recip{i}")
                        for i in range(NQB)]

            # ---- resident weights; wv first (needed immediately) ----
            wv_t = wpool.tile([P, DT, E], bf16, name="wv_t")
            for t in range(DT):
                nc.scalar.dma_start(wv_t[:, t, :], wv_r[:, t, :])
            wk_t = wpool.tile([P, DT, E], bf16, name="wk_t")
            wq_t = wpool.tile([P, DT, E], bf16, name="wq_t")
            ow_t = wpool.tile([P, ET, D], bf16, name="ow_t")
            # background loads on the vector queue (needed at kp / qb time)
            for h in range(DT // 2):
                nc.gpsimd.dma_start(wk_t[:, 2 * h:2 * h + 2, :],
                                    wk_r[:, 2 * h:2 * h + 2, :])
            qT_sb = bigres.tile([P, DT, QSH], bf16, name="qT_sb")
            for h in range(DT // 2):
                nc.gpsimd.dma_start(qT_sb[:, 2 * h:2 * h + 2, :],
                                    qT_r[:, 2 * h:2 * h + 2, :])
            for h in range(DT // 2):
                nc.gpsimd.dma_start(wq_t[:, 2 * h:2 * h + 2, :],
                                    wq_r[:, 2 * h:2 * h + 2, :])
            for h in range(ET // 2):
                nc.gpsimd.dma_start(ow_t[:, 2 * h:2 * h + 2, :],
                                    ow_r[:, 2 * h:2 * h + 2, :])

            # ---- resident vp / kp ----
            vp = bigres.tile([P, KT, E], bf16, name="vp")
            kp = bigres.tile([P, ET, SK], bf16, name="kp")

            # ---- phase VP: vp [SK, E] (no bias; folded into ob) ----
            with tc.tile_pool(name="vp_st", bufs=6) as vst, \
                 tc.tile_pool(name="vp_ps", bufs=4, space="PSUM") as vps:
                for m in range(KT):
                    lhs_t = vst.tile([P, DT, P], bf16, tag="vT_s", name=f"vT_{m}")
                    nc.sync.dma_start(lhs_t[:], vT_r[:, :, m * P:(m + 1) * P])
                    for n in range(E // 512):
                        ps = vps.tile([P, 512], f32, tag="ps", name=f"vpps_{m}_{n}")
                        for t in range(DT):
                            mm(ps[:], lhs_t[:, t, :],
                               wv_t[:, t, n * 512:(n + 1) * 512],
                               t == 0, t == DT - 1)
                        nc.scalar.activation(vp[:, m, n * 512:(n + 1) * 512],
                                             ps[:], AF.Identity)

            # ---- phase KP: kp^T [E, SK] + bk ----
            with tc.tile_pool(name="kp_st", bufs=2) as kst, \
                 tc.tile_pool(name="kp_ps", bufs=1, space="PSUM") as kps:
                for n in range(NKC):
                    rhs_t = kst.tile([P, DT, KC], bf16, tag="kT_s", name=f"kT_{n}")
                    nc.sync.dma_start(rhs_t[:], kT_r[:, :, n * KC:(n + 1) * KC])
                    for m in range(ET):
                        ps = kps.tile([P, KC], f32, tag=f"ps{m}",
                                      name=f"kpps_{n}_{m}")
                        for t in range(DT):
                            mm(ps[:], wk_t[:, t, m * P:(m + 1) * P],
                               rhs_t[:, t, :], t == 0, t == DT - 1)
                        nc.scalar.activation(kp[:, m, n * KC:(n + 1) * KC],
                                             ps[:], AF.Identity,
                                             bias=bk_t[:, m:m + 1])

            # ---- attention + fused out projection, per q-block ----
            with tc.tile_pool(name="qp_sb", bufs=1) as qppool, \
                 tc.tile_pool(name="exp_sb", bufs=1) as exppool, \
                 tc.tile_pool(name="ctx_sb", bufs=1) as ctxpool, \
                 tc.tile_pool(name="out_sb", bufs=4) as outpool:
                for qb in range(NQB):
                    q0 = qb * QB

                    # -- qp^T for this q block --
                    qp = qppool.tile([P, ET, QB], bf16, tag="qp", name=f"qp{qb}")
                    with tc.tile_pool(name=f"qp_ps{qb}", bufs=1,
                                      space="PSUM") as php:
                        for m in range(ET):
                            ps = php.tile([P, QB], f32, tag=f"ps{m % 2}",
                                          name=f"qpps{qb}_{m}")
                            for t in range(DT):
                                mm(ps[:], wq_t[:, t, m * P:(m + 1) * P],
                                   qT_sb[:, t, q0:q0 + QB], t == 0, t == DT - 1)
                            nc.scalar.activation(qp[:, m, :], ps[:], AF.Identity,
                                                 bias=bq_t[:, m:m + 1],
                                                 scale=ISCALE)

                    # -- logits + exp + softmax sum --
                    expT = exppool.tile([P, KT, QB], bf16, tag="exp",
                                        name=f"exp{qb}")
                    with tc.tile_pool(name=f"lg_ps{qb}", bufs=1,
                                      space="PSUM") as php, \
                         tc.tile_pool(name=f"s_ps{qb}", bufs=1,
                                      space="PSUM") as sphp:
                        s_ps = sphp.tile([P, QB], f32, name=f"sps{qb}")
                        for kb in range(KT):
                            ps = php.tile([P, QB], f32, tag=f"ps{kb % 3}",
                                          name=f"lgps{qb}_{kb}")
                            for e in range(ET):
                                mm(ps[:], kp[:, e, kb * P:(kb + 1) * P],
                                   qp[:, e, :], e == 0, e == ET - 1)
                            nc.scalar.activation(expT[:, kb, :], ps[:], AF.Exp,
                                                 bias=mask_t[:, kb:kb + 1])
                            mm(s_ps[:], ones_t[:], expT[:, kb, :],
                               kb == 0, kb == KT - 1)
                        nc.vector.reciprocal(recip_ts[qb][:], s_ps[:])

                    # -- ctx accumulation + normalize --
                    ctx_sb = ctxpool.tile([P, ET, QB], bf16, tag="ctx",
                                          name=f"ctx{qb}")
                    with tc.tile_pool(name=f"ctx_ps{qb}", bufs=1,
                                      space="PSUM") as php:
                        for e in range(ET):
                            ps = php.tile([P, QB], f32, tag=f"ps{e % 3}",
                                          name=f"ctxps{qb}_{e}")
                            for kb in range(KT):
                                mm(ps[:], vp[:, kb, e * P:(e + 1) * P],
                                   expT[:, kb, :], kb == 0, kb == KT - 1)
                            nc.vector.tensor_mul(ctx_sb[:, e, :], ps[:],
                                                 recip_ts[qb][:])

                    # -- fused out projection: out[q,:] = ctx @ ow + ob_eff --
                    with tc.tile_pool(name=f"out_ps{qb}", bufs=1,
                                      space="PSUM") as php:
                        for nd in range(D // DNB):
                            for mq in range(MQ):
                                ps = php.tile([P, DNB], f32,
                                              tag=f"ps{(nd * MQ + mq) % 3}",
                                              name=f"ops{qb}_{nd}_{mq}")
                                for e in range(ET):
                                    mm(ps[:],
                                       ctx_sb[:, e, mq * P:(mq + 1) * P],
                                       ow_t[:, e, nd * DNB:(nd + 1) * DNB],
                                       e == 0, e == ET - 1)
                                ot = outpool.tile([P, DNB], f32, tag="ot",
                                                  name=f"ot{qb}_{nd}_{mq}")
                                nc.vector.tensor_add(
                                    ot[:], ps[:],
                                    ob_t[:, nd * DNB:(nd + 1) * DNB])
                                nc.gpsimd.dma_start(
                                    out[q0 + mq * P: q0 + (mq + 1) * P,
                                        nd * DNB:(nd + 1) * DNB], ot[:])

    nc.compile()
    return nc


def make_in_maps(v, k, q, mask, wq_w, wq_b, wk_w, wk_b, wv_w, wv_b, out_w, out_b,
                 n_cores=8, D=1024, E=1024, SK=2048, QSH=1024):
    """Host-side shard + layout prep (pure data movement + bias folding)."""
    ET = E // P
    KT = SK // P
    f = np.float32
    ISCALE = 1.0 / float(np.sqrt(E))
    wq_bf = np.ascontiguousarray(np.asarray(wq_w, f).astype(BF16))
    wk_bf = np.ascontiguousarray(np.asarray(wk_w, f).astype(BF16))
    wv_bf = np.ascontiguousarray(np.asarray(wv_w, f).astype(BF16))
    ow_bf = np.ascontiguousarray(np.asarray(out_w, f).astype(BF16))
    bq_col = np.ascontiguousarray(
        (np.asarray(wq_b, f) * ISCALE).reshape(ET, P).T)
    bk_col = np.ascontiguousarray(np.asarray(wk_b, f).reshape(ET, P).T)
    # softmax rows sum to 1, so the vp bias adds exactly bv @ ow to every row
    ob_eff = np.asarray(out_b, f) + np.asarray(wv_b, f) @ np.asarray(out_w, f)
    ob_bc = np.ascontiguousarray(np.broadcast_to(ob_eff, (P, len(out_b))))
    ones_arr = np.ones((P, P), BF16)
    in_maps = []
    for c in range(n_cores):
        b, h = divmod(c, 2)
        qTc = np.ascontiguousarray(
            np.asarray(q[b, h * QSH:(h + 1) * QSH, :], f).T.astype(BF16))
        kTc = np.ascontiguousarray(np.asarray(k[b], f).T.astype(BF16))
        vTc = np.ascontiguousarray(np.asarray(v[b], f).T.astype(BF16))
        mc = np.ascontiguousarray(
            (np.asarray(mask[b, 0], f) * NEG).reshape(KT, P).T)
        in_maps.append(dict(qT=qTc, kT=kTc, vT=vTc, mask_cols=mc,
                            ones_d=ones_arr,
                            wq=wq_bf, wk=wk_bf, wv=wv_bf, ow=ow_bf,
                            bq_col=bq_col, bk_col=bk_col, ob_bc=ob_bc))
    return in_maps


_NC_CACHE = {}


def kernel(v, k, q, mask, wq_w, wq_b, wk_w, wk_b, wv_w, wv_b, out_w, out_b):
    import os
    from concourse.bass_utils import run_bass_kernel_spmd

    B, S, D = 4, 2048, 1024
    E, QSH = 1024, 1024
    if "nc" not in _NC_CACHE:
        _NC_CACHE["nc"] = build_nc(D=D, E=E, SK=S, QSH=QSH, QB=512)
    nc = _NC_CACHE["nc"]

    in_maps = make_in_maps(v, k, q, mask, wq_w, wq_b, wk_w, wk_b, wv_w, wv_b,
                           out_w, out_b, n_cores=8, D=D, E=E, SK=S, QSH=QSH)
    trace = bool(int(os.environ.get("BASS_KERNEL_TRACE", "0")))
    res = run_bass_kernel_spmd(nc, in_maps, core_ids=list(range(8)), trace=trace)
    if trace:
        print(f"HW exec time: {res.exec_time_ns} ns")
        _NC_CACHE["last_exec_time_ns"] = res.exec_time_ns
        _NC_CACHE["last_trace"] = res.instructions_and_trace

    outp = np.empty((B, S, D), np.float32)
    for c in range(8):
        b, h = divmod(c, 2)
        outp[b, h * QSH:(h + 1) * QSH, :] = res.results[c]["out"]
    return outp
